# revision 2
# baseline (speedup 1.0000x reference)
"""GCN-VAE encoder on 8 Trainium2 NeuronCores — v2.

Design (vs v1 baseline): the sparse aggregation's per-edge dma_gather is
replaced by host-side per-edge halo exchange: between launches the host lays
out, per core, the message table sup[src_e] in (slot, chunk, partition) order
so each device reads it as a single sequential stream at full DMA bus rate
(v1's 256-byte gather rows paid a 2x sub-512B descriptor penalty and per-row
descriptor costs; v1 Pool desc-gen is also gone). Segment-sum stays a PE
matmul acc += S^T @ G with one-hot S built from EMETA; S-builds alternate
DVE/Pool to keep both under the DMA roofline. The L2 dense tail computes
h1^T directly (accT = G^T-slices @ S) so h1 @ [W2|W3] needs no transposes.

Launches:
  L1: support1_shard = x_shard @ W1               (node-window pipelined)
  L2: h1T = relu(spmmT(SUPX2)); s23 = h1 @ W23    (per dst-window)
  L3: [mu|logvar] = relu(spmm(SUPX3)); z = eps*exp(logvar)+mu
"""

import sys

for _p in ("/opt/trn_rl_repo", "/root/.axon_site/_ro/trn_rl_repo"):
    if _p not in sys.path:
        sys.path.append(_p)

import numpy as np

import concourse.mybir as mybir
import concourse.tile as tile
from concourse import bacc
from concourse.bass_utils import run_bass_kernel_spmd

# ---- problem constants (hardcoded per harness contract) ----
N, E, F_IN, H1, H2 = 50000, 1600000, 512, 256, 64
H23 = 2 * H2                      # concat(mu, logvar) feature width
M = 8                             # cores
NSH = N // M                      # nodes per core
P = 128                           # partitions / window size / edge chunk
NWIN = (NSH + P - 1) // P         # dst windows per core (49)
KCH = F_IN // P                   # k-chunks for layer-1 matmul (4)
NPAIR = (NWIN + 1) // 2

f32 = mybir.dt.float32
f16 = mybir.dt.float16

_PROG_CACHE: dict = {}


# ---------------------------------------------------------------- host prep
def _prep_edges(edge_src, edge_dst, edge_weight):
    """Partition edges by dst shard, window-sort, slot-balance; emit per-core
    (SRC slot-ordered edge sources, EMETA one-hot build scalars, perm)."""
    edge_src = np.asarray(edge_src).astype(np.int64)
    edge_dst = np.asarray(edge_dst).astype(np.int64)
    edge_weight = np.asarray(edge_weight).astype(np.float32)

    percore = []
    cnts = np.zeros((M, NWIN), np.int64)
    perms = []
    for m in range(M):
        sel = (edge_dst >= m * NSH) & (edge_dst < (m + 1) * NSH)
        d = edge_dst[sel] - m * NSH
        s = edge_src[sel]
        w = edge_weight[sel]
        win0 = d >> 7
        # slot-balance: rank windows by edge count so slot j holds each
        # core's j-th-largest window -> cross-core max padding shrinks
        wcnt = np.bincount(win0, minlength=NWIN)
        perm = np.argsort(-wcnt, kind="stable")           # slot -> window
        inv = np.empty(NWIN, np.int64)
        inv[perm] = np.arange(NWIN)                       # window -> slot
        perms.append(perm)
        win = inv[win0]                                   # slot index per edge
        order = np.argsort(win, kind="stable")
        d, s, w, win = d[order], s[order], w[order], win[order]
        dloc = d - (perm[win] << 7)                       # dst_local in window
        cnts[m] = np.bincount(win, minlength=NWIN)
        percore.append((dloc, s, w, win))

    ncw = np.maximum(1, -(-cnts.max(axis=0) // P))        # chunks per slot
    offs = np.concatenate([[0], np.cumsum(ncw)])
    totc = int(offs[-1])

    shards = []
    for m in range(M):
        dloc, s, w, win = percore[m]
        DSTL = np.full((P, totc), -1.0, np.float32)
        WGT = np.zeros((P, totc), np.float32)
        SRC = np.full(totc * P, -1, np.int64)
        starts = np.concatenate([[0], np.cumsum(cnts[m])])[:-1]
        j = np.arange(len(dloc)) - starts[win]            # rank within slot
        rows = j % P
        cols = offs[win] + j // P
        DSTL[rows, cols] = dloc
        WGT[rows, cols] = w
        SRC[cols * P + rows] = s
        EMETA = np.empty((P, 2 * totc), np.float32)
        for t in range(NWIN):
            o, nw = int(offs[t]), int(ncw[t])
            EMETA[:, 2 * o:2 * o + nw] = DSTL[:, o:o + nw]
            EMETA[:, 2 * o + nw:2 * (o + nw)] = WGT[:, o:o + nw]
        shards.append((SRC, EMETA, perms[m]))

    key = tuple(int(v) for v in ncw)
    meta = (tuple(int(v) for v in ncw), offs, totc)
    return key, meta, shards


def _expand(sup_full, SRC, H):
    """[N, H] table -> [128, totc, H] slot-ordered per-edge message stream."""
    out = np.zeros((SRC.shape[0], H), sup_full.dtype)
    valid = SRC >= 0
    out[valid] = sup_full[SRC[valid]]
    return np.ascontiguousarray(
        out.reshape(-1, P, H).transpose(1, 0, 2))


# ------------------------------------------------------------- bass builders
def _mk_nc():
    return bacc.Bacc("TRN2", target_bir_lowering=False, debug=False)


def _build_l1():
    """s1[128, NWIN, 256] = x_shard @ W1, span-pipelined."""
    nc = _mk_nc()
    xL = nc.dram_tensor("xL", [P, NWIN * P, KCH], f16, kind="ExternalInput")
    W1 = nc.dram_tensor("W1", [F_IN, H1], f16, kind="ExternalInput")
    s1 = nc.dram_tensor("s1", [P, NWIN, H1], f16, kind="ExternalOutput")

    SPAN = 7                      # windows per pipeline span
    NSPAN = -(-NWIN // SPAN)
    with tile.TileContext(nc) as tc:
        with tc.tile_pool(name="const", bufs=1) as cpool, \
             tc.tile_pool(name="xin", bufs=4) as xpool, \
             tc.tile_pool(name="out", bufs=3) as opool, \
             tc.tile_pool(name="psum", bufs=4, space="PSUM") as psum:
            w1c = cpool.tile([P, KCH, H1], f16)
            nc.sync.dma_start(out=w1c[:],
                              in_=W1[:].rearrange("(k p) n -> p k n", p=P))
            for sp in range(NSPAN):
                a = sp * SPAN * P
                nsw = min(SPAN, NWIN - sp * SPAN)
                xsp = xpool.tile([P, SPAN * P, KCH], f16, tag="xsp")
                nc.sync.dma_start(out=xsp[:, :nsw * P, :],
                                  in_=xL[:, a:a + nsw * P, :])
                osp = opool.tile([P, SPAN, H1], f16, tag="osp")
                for t in range(nsw):
                    acc = psum.tile([P, H1], f32, space="PSUM", tag="acc")
                    for k in range(KCH):
                        nc.tensor.matmul(
                            out=acc[:],
                            lhsT=xsp[:, t * P:(t + 1) * P, k],
                            rhs=w1c[:, k, :],
                            start=(k == 0), stop=(k == KCH - 1))
                    nc.vector.tensor_copy(out=osp[:, t, :], in_=acc[:])
                nc.scalar.dma_start(out=s1[:, sp * SPAN:sp * SPAN + nsw, :],
                                    in_=osp[:, :nsw, :])
    nc.compile()
    return nc


def _spmm_windows(nc, spool, psum, gpool, metafull, supx, iota_t, meta,
                  H, per_chunk_mm, per_window_out):
    """Shared spmm loop: per slot-window load the expanded message stream,
    build one-hot S per chunk (alternating DVE/Pool), call per_chunk_mm for
    the PE accumulation, then per_window_out."""
    ncw, offs, totc = meta
    sb = [0]

    for win in range(NWIN):
        nw = ncw[win]
        off = int(offs[win])
        G = gpool.tile([P, nw, H], f16, tag="G")
        nc.sync.dma_start(out=G[:], in_=supx[:, off:off + nw, :])
        accs = per_chunk_mm(None, None, -1, nw)    # fresh psum tiles
        for c in range(nw):
            S = spool.tile([P, P], f16, tag="S")
            eng = nc.vector if (sb[0] % 3) < 2 else nc.gpsimd
            sb[0] += 1
            eng.tensor_scalar(
                out=S[:], in0=iota_t[:],
                scalar1=metafull[:, 2 * off + c:2 * off + c + 1],
                scalar2=metafull[:, 2 * off + nw + c:2 * off + nw + c + 1],
                op0=mybir.AluOpType.is_equal, op1=mybir.AluOpType.mult)
            per_chunk_mm(G, S, c, nw, accs)
        per_window_out(win, accs)


def _build_l2(meta):
    """h1T = relu(spmmT(sup1 stream)); s23 = h1 @ W23 (transpose-free)."""
    ncw, offs, totc = meta
    nc = _mk_nc()
    supx = nc.dram_tensor("supx", [P, totc, H1], f16, kind="ExternalInput")
    emeta = nc.dram_tensor("emeta", [P, 2 * totc], f32, kind="ExternalInput")
    W23 = nc.dram_tensor("W23", [H1, H23], f16, kind="ExternalInput")
    iota_h = nc.dram_tensor("iota", [P, P], f16, kind="ExternalInput")
    s23 = nc.dram_tensor("s23", [P, NPAIR, 2, H23], f16, kind="ExternalOutput")

    KC2 = H1 // P                 # 2 feature k-chunks
    with tile.TileContext(nc) as tc:
        with tc.tile_pool(name="const", bufs=1) as cpool, \
             tc.tile_pool(name="spool", bufs=12) as spool, \
             tc.tile_pool(name="gpool", bufs=4) as gpool, \
             tc.tile_pool(name="hpool", bufs=3) as hpool, \
             tc.tile_pool(name="opool", bufs=3) as opool, \
             tc.tile_pool(name="psum", bufs=3, space="PSUM") as psum, \
             tc.tile_pool(name="psum2", bufs=2, space="PSUM") as psum2:
            iota_t = cpool.tile([P, P], f16)
            nc.scalar.dma_start(out=iota_t[:], in_=iota_h[:])
            metafull = cpool.tile([P, 2 * totc], f32)
            nc.scalar.dma_start(out=metafull[:], in_=emeta[:])
            w23c = cpool.tile([P, KC2, H23], f16)
            nc.scalar.dma_start(out=w23c[:],
                                in_=W23[:].rearrange("(k p) n -> p k n", p=P))

            opair_box = [None]

            def per_chunk_mm(G, S, c, nw, accs=None):
                if c == -1:
                    return [psum.tile([P, P], f32, space="PSUM",
                                      name=f"accT{k}", tag=f"accT{k}")
                            for k in range(KC2)]
                for k in range(KC2):
                    nc.tensor.matmul(
                        out=accs[k][:],
                        lhsT=G[:, c, k * P:(k + 1) * P],
                        rhs=S[:],
                        start=(c == 0), stop=(c == nw - 1))

            def per_window_out(win, accs):
                h1T = hpool.tile([P, KC2, P], f16, tag="h1T")
                for k in range(KC2):
                    nc.scalar.activation(out=h1T[:, k, :], in_=accs[k][:],
                                         func=mybir.ActivationFunctionType.Relu)
                ps23 = psum2.tile([P, H23], f32, space="PSUM", tag="ps23")
                for k in range(KC2):
                    nc.tensor.matmul(
                        out=ps23[:],
                        lhsT=h1T[:, k, :],
                        rhs=w23c[:, k, :],
                        start=(k == 0), stop=(k == KC2 - 1))
                if win % 2 == 0:
                    opair_box[0] = opool.tile([P, 2, H23], f16, name="opair", tag="opair")
                opair = opair_box[0]
                nc.scalar.activation(out=opair[:, win % 2, :], in_=ps23[:],
                                     func=mybir.ActivationFunctionType.Copy)
                pb = win // 2
                if win % 2 == 1:
                    nc.scalar.dma_start(out=s23[:, pb, :, :], in_=opair[:])
                elif win == NWIN - 1:
                    nc.scalar.dma_start(out=s23[:, pb, 0, :],
                                        in_=opair[:, 0, :])

            _spmm_windows(nc, spool, psum, gpool, metafull, supx, iota_t,
                          meta, H1, per_chunk_mm, per_window_out)
    nc.compile()
    return nc


def _build_l3(meta):
    """[mu|logvar] = relu(spmm(sup23 stream)); z = eps*exp(logvar)+mu."""
    ncw, offs, totc = meta
    nc = _mk_nc()
    supx = nc.dram_tensor("supx", [P, totc, H23], f16, kind="ExternalInput")
    emeta = nc.dram_tensor("emeta", [P, 2 * totc], f32, kind="ExternalInput")
    iota_h = nc.dram_tensor("iota", [P, P], f16, kind="ExternalInput")
    epss = nc.dram_tensor("epss", [P, NWIN * H2], f16, kind="ExternalInput")
    out3 = nc.dram_tensor("out3", [P, NPAIR, 2, 3 * H2], f16,
                          kind="ExternalOutput")

    with tile.TileContext(nc) as tc:
        with tc.tile_pool(name="const", bufs=1) as cpool, \
             tc.tile_pool(name="spool", bufs=12) as spool, \
             tc.tile_pool(name="gpool", bufs=4) as gpool, \
             tc.tile_pool(name="expool", bufs=3) as expool, \
             tc.tile_pool(name="opool", bufs=3) as opool, \
             tc.tile_pool(name="psum", bufs=4, space="PSUM") as psum:
            iota_t = cpool.tile([P, P], f16)
            nc.scalar.dma_start(out=iota_t[:], in_=iota_h[:])
            metafull = cpool.tile([P, 2 * totc], f32)
            nc.scalar.dma_start(out=metafull[:], in_=emeta[:])
            epsfull = cpool.tile([P, NWIN, H2], f16)
            nc.scalar.dma_start(out=epsfull[:], in_=epss[:])

            opair_box = [None]
            pending = []

            def per_chunk_mm(G, S, c, nw, accs=None):
                if c == -1:
                    return [psum.tile([P, H23], f32, space="PSUM",
                                      name="acc", tag="acc")]
                nc.tensor.matmul(
                    out=accs[0][:],
                    lhsT=S[:],
                    rhs=G[:, c, :],
                    start=(c == 0), stop=(c == nw - 1))

            def tail(win, op, ex_t):
                # deferred one window so the DVE queue never waits on Act
                q = win % 2
                nc.vector.tensor_mul(out=op[:, q, 0:H2], in0=ex_t[:],
                                     in1=epsfull[:, win, :])
                nc.vector.tensor_add(out=op[:, q, 0:H2], in0=op[:, q, 0:H2],
                                     in1=op[:, q, H2:H23])
                pb = win // 2
                if win % 2 == 1:
                    nc.scalar.dma_start(out=out3[:, pb, :, :], in_=op[:])
                elif win == NWIN - 1:
                    nc.scalar.dma_start(out=out3[:, pb, 0, :], in_=op[:, 0, :])

            def per_window_out(win, accs):
                acc = accs[0]
                if win % 2 == 0:
                    opair_box[0] = opool.tile([P, 2, 3 * H2], f16,
                                              name="opair", tag="opair")
                op = opair_box[0]
                q = win % 2
                # o = [z | mu | logvar]
                nc.scalar.activation(out=op[:, q, H2:3 * H2], in_=acc[:],
                                     func=mybir.ActivationFunctionType.Relu)
                ex_t = expool.tile([P, H2], f16, tag="ex")
                nc.scalar.activation(out=ex_t[:], in_=op[:, q, H23:3 * H2],
                                     func=mybir.ActivationFunctionType.Exp)
                pending.append((win, op, ex_t))
                if len(pending) > 1:
                    tail(*pending.pop(0))
                if win == NWIN - 1:
                    while pending:
                        tail(*pending.pop(0))

            _spmm_windows(nc, spool, psum, gpool, metafull, supx, iota_t,
                          meta, H23, per_chunk_mm, per_window_out)
    nc.compile()
    return nc


def _get_progs(key, meta):
    if key not in _PROG_CACHE:
        _PROG_CACHE[key] = (_build_l1(), _build_l2(meta), _build_l3(meta))
    return _PROG_CACHE[key]


# ------------------------------------------------------------------- kernel
def _run_spmd(nc, in_maps, tries=4):
    """run_bass_kernel_spmd with retries: the shared device pool occasionally
    needs a few minutes to recover a wedged worker."""
    import time
    for attempt in range(tries):
        try:
            return run_bass_kernel_spmd(nc, in_maps, core_ids=list(range(M)))
        except Exception:
            if attempt == tries - 1:
                raise
            time.sleep(90)


def kernel(x, W1, W2, W3, edge_weight, eps, edge_src, edge_dst):
    x = np.asarray(x, np.float32)
    W1 = np.asarray(W1, np.float32)
    W23 = np.concatenate([np.asarray(W2, np.float32),
                          np.asarray(W3, np.float32)], axis=1)
    eps = np.asarray(eps, np.float32)

    key, meta, eshards = _prep_edges(edge_src, edge_dst, edge_weight)
    ncw, offs, totc = meta
    nc1, nc2, nc3 = _get_progs(key, meta)

    iota = np.broadcast_to(
        np.arange(P, dtype=np.float16)[None, :], (P, P))

    # ---- L1: support1 shards
    NPAD = NWIN * P
    in1 = []
    for m in range(M):
        xs = np.zeros((NPAD, F_IN), np.float16)
        xs[:NSH] = x[m * NSH:(m + 1) * NSH].astype(np.float16)
        xLm = np.ascontiguousarray(
            xs.reshape(NPAD, KCH, P).transpose(2, 0, 1))   # [128, NPAD, KCH]
        in1.append({"xL": xLm, "W1": W1.astype(np.float16)})
    r1 = _run_spmd(nc1, in1)
    sup1 = np.concatenate(
        [r1.results[m]["s1"].transpose(1, 0, 2).reshape(NPAD, H1)[:NSH]
         for m in range(M)], axis=0)                       # [N, 256] f16

    def unslot(block, m, H):
        """[NWIN*P, H] slot-blocked -> [NSH, H] node-ordered for core m."""
        perm = eshards[m][2]
        out = np.empty((NSH, H), block.dtype)
        for j in range(NWIN):
            wj = int(perm[j])
            r = min(P, NSH - wj * P)
            out[wj * P:wj * P + r] = block[j * P:j * P + r]
        return out

    def toslot(arr, m):
        """[NSH, H] node-ordered -> [NWIN*P, H] slot-blocked for core m."""
        perm = eshards[m][2]
        out = np.zeros((NWIN * P, arr.shape[1]), arr.dtype)
        for j in range(NWIN):
            wj = int(perm[j])
            r = min(P, NSH - wj * P)
            out[j * P:j * P + r] = arr[wj * P:wj * P + r]
        return out

    # ---- L2: h1 + support23 shards
    in2 = [{"supx": _expand(sup1, eshards[m][0], H1),
            "emeta": eshards[m][1],
            "W23": W23.astype(np.float16), "iota": np.asarray(iota)}
           for m in range(M)]
    r2 = _run_spmd(nc2, in2)
    sup23_parts = []
    for m in range(M):
        pr = r2.results[m]["s23"]                  # [128, NPAIR, 2, 128]
        sl = pr.transpose(1, 2, 0, 3).reshape(NPAIR * 2 * P, H23)[:NWIN * P]
        sup23_parts.append(unslot(sl, m, H23))
    sup23 = np.concatenate(sup23_parts, axis=0)    # [N, 128] f16

    # ---- L3: mu, logvar, z shards
    in3 = [{"supx": _expand(sup23, eshards[m][0], H23),
            "emeta": eshards[m][1], "iota": np.asarray(iota),
            "epss": np.ascontiguousarray(
                toslot(eps[m * NSH:(m + 1) * NSH].astype(np.float16), m)
                .reshape(NWIN, P, H2).transpose(1, 0, 2).reshape(P, NWIN * H2))}
           for m in range(M)]
    r3 = _run_spmd(nc3, in3)
    outs = []
    for m in range(M):
        pr = r3.results[m]["out3"]                 # [128, NPAIR, 2, 192]
        sl = pr.transpose(1, 2, 0, 3).reshape(NPAIR * 2 * P, 3 * H2)[:NWIN * P]
        outs.append(unslot(sl, m, 3 * H2))
    full = np.concatenate(outs, axis=0).astype(np.float32)
    z, mu, logvar = full[:, 0:H2], full[:, H2:H23], full[:, H23:3 * H2]
    return (np.ascontiguousarray(z), np.ascontiguousarray(mu),
            np.ascontiguousarray(logvar))


# revision 3
# speedup vs baseline: 1.0089x; 1.0089x over previous
"""GCN-VAE encoder on 8 Trainium2 NeuronCores — v2.

Design (vs v1 baseline): the sparse aggregation's per-edge dma_gather is
replaced by host-side per-edge halo exchange: between launches the host lays
out, per core, the message table sup[src_e] in (slot, chunk, partition) order
so each device reads it as a single sequential stream at full DMA bus rate
(v1's 256-byte gather rows paid a 2x sub-512B descriptor penalty and per-row
descriptor costs; v1 Pool desc-gen is also gone). Segment-sum stays a PE
matmul acc += S^T @ G with one-hot S built from EMETA; S-builds alternate
DVE/Pool to keep both under the DMA roofline. The L2 dense tail computes
h1^T directly (accT = G^T-slices @ S) so h1 @ [W2|W3] needs no transposes.

Launches:
  L1: support1_shard = x_shard @ W1               (node-window pipelined)
  L2: h1T = relu(spmmT(SUPX2)); s23 = h1 @ W23    (per dst-window)
  L3: [mu|logvar] = relu(spmm(SUPX3)); z = eps*exp(logvar)+mu
"""

import sys

for _p in ("/opt/trn_rl_repo", "/root/.axon_site/_ro/trn_rl_repo"):
    if _p not in sys.path:
        sys.path.append(_p)

import numpy as np

import concourse.mybir as mybir
import concourse.tile as tile
from concourse import bacc
from concourse.bass_utils import run_bass_kernel_spmd

# ---- problem constants (hardcoded per harness contract) ----
N, E, F_IN, H1, H2 = 50000, 1600000, 512, 256, 64
H23 = 2 * H2                      # concat(mu, logvar) feature width
M = 8                             # cores
NSH = N // M                      # nodes per core
P = 128                           # partitions / window size / edge chunk
NWIN = (NSH + P - 1) // P         # dst windows per core (49)
KCH = F_IN // P                   # k-chunks for layer-1 matmul (4)
NPAIR = (NWIN + 1) // 2

f32 = mybir.dt.float32
f16 = mybir.dt.float16

_PROG_CACHE: dict = {}


# ---------------------------------------------------------------- host prep
def _prep_edges(edge_src, edge_dst, edge_weight):
    """Partition edges by dst shard, window-sort, slot-balance; emit per-core
    (SRC slot-ordered edge sources, EMETA one-hot build scalars, perm)."""
    edge_src = np.asarray(edge_src).astype(np.int64)
    edge_dst = np.asarray(edge_dst).astype(np.int64)
    edge_weight = np.asarray(edge_weight).astype(np.float32)

    percore = []
    cnts = np.zeros((M, NWIN), np.int64)
    perms = []
    for m in range(M):
        sel = (edge_dst >= m * NSH) & (edge_dst < (m + 1) * NSH)
        d = edge_dst[sel] - m * NSH
        s = edge_src[sel]
        w = edge_weight[sel]
        win0 = d >> 7
        # slot-balance: rank windows by edge count so slot j holds each
        # core's j-th-largest window -> cross-core max padding shrinks
        wcnt = np.bincount(win0, minlength=NWIN)
        perm = np.argsort(-wcnt, kind="stable")           # slot -> window
        inv = np.empty(NWIN, np.int64)
        inv[perm] = np.arange(NWIN)                       # window -> slot
        perms.append(perm)
        win = inv[win0]                                   # slot index per edge
        order = np.argsort(win, kind="stable")
        d, s, w, win = d[order], s[order], w[order], win[order]
        dloc = d - (perm[win] << 7)                       # dst_local in window
        cnts[m] = np.bincount(win, minlength=NWIN)
        percore.append((dloc, s, w, win))

    ncw = np.maximum(1, -(-cnts.max(axis=0) // P))        # chunks per slot
    offs = np.concatenate([[0], np.cumsum(ncw)])
    totc = int(offs[-1])

    shards = []
    for m in range(M):
        dloc, s, w, win = percore[m]
        DSTL = np.full((P, totc), -1.0, np.float32)
        WGT = np.zeros((P, totc), np.float32)
        SRC = np.full(totc * P, -1, np.int64)
        starts = np.concatenate([[0], np.cumsum(cnts[m])])[:-1]
        j = np.arange(len(dloc)) - starts[win]            # rank within slot
        rows = j % P
        cols = offs[win] + j // P
        DSTL[rows, cols] = dloc
        WGT[rows, cols] = w
        SRC[cols * P + rows] = s
        EMETA = np.empty((P, 2 * totc), np.float16)
        for t in range(NWIN):
            o, nw = int(offs[t]), int(ncw[t])
            EMETA[:, 2 * o:2 * o + nw] = DSTL[:, o:o + nw]
            EMETA[:, 2 * o + nw:2 * (o + nw)] = WGT[:, o:o + nw]
        shards.append((SRC, EMETA, perms[m]))

    key = tuple(int(v) for v in ncw)
    meta = (tuple(int(v) for v in ncw), offs, totc)
    return key, meta, shards


def _expand(sup_full, SRC, H):
    """[N, H] table -> [128, totc, H] slot-ordered per-edge message stream."""
    out = np.zeros((SRC.shape[0], H), sup_full.dtype)
    valid = SRC >= 0
    out[valid] = sup_full[SRC[valid]]
    return np.ascontiguousarray(
        out.reshape(-1, P, H).transpose(1, 0, 2))


# ------------------------------------------------------------- bass builders
def _mk_nc():
    return bacc.Bacc("TRN2", target_bir_lowering=False, debug=False)


def _build_l1():
    """s1[128, NWIN, 256] = x_shard @ W1, span-pipelined."""
    nc = _mk_nc()
    xL = nc.dram_tensor("xL", [P, NWIN * P, KCH], f16, kind="ExternalInput")
    W1 = nc.dram_tensor("W1", [F_IN, H1], f16, kind="ExternalInput")
    s1 = nc.dram_tensor("s1", [P, NWIN, H1], f16, kind="ExternalOutput")

    SPAN = 7                      # windows per pipeline span
    NSPAN = -(-NWIN // SPAN)
    with tile.TileContext(nc) as tc:
        with tc.tile_pool(name="const", bufs=1) as cpool, \
             tc.tile_pool(name="xin", bufs=4) as xpool, \
             tc.tile_pool(name="out", bufs=3) as opool, \
             tc.tile_pool(name="psum", bufs=4, space="PSUM") as psum:
            w1c = cpool.tile([P, KCH, H1], f16)
            nc.sync.dma_start(out=w1c[:],
                              in_=W1[:].rearrange("(k p) n -> p k n", p=P))
            for sp in range(NSPAN):
                a = sp * SPAN * P
                nsw = min(SPAN, NWIN - sp * SPAN)
                xsp = xpool.tile([P, SPAN * P, KCH], f16, tag="xsp")
                nc.sync.dma_start(out=xsp[:, :nsw * P, :],
                                  in_=xL[:, a:a + nsw * P, :])
                osp = opool.tile([P, SPAN, H1], f16, tag="osp")
                for t in range(nsw):
                    acc = psum.tile([P, H1], f32, space="PSUM", tag="acc")
                    for k in range(KCH):
                        nc.tensor.matmul(
                            out=acc[:],
                            lhsT=xsp[:, t * P:(t + 1) * P, k],
                            rhs=w1c[:, k, :],
                            start=(k == 0), stop=(k == KCH - 1))
                    nc.vector.tensor_copy(out=osp[:, t, :], in_=acc[:])
                nc.scalar.dma_start(out=s1[:, sp * SPAN:sp * SPAN + nsw, :],
                                    in_=osp[:, :nsw, :])
    nc.compile()
    return nc


def _spmm_windows(nc, spool, psum, gpool, metafull, supx, iota_t, meta,
                  H, per_chunk_mm, per_window_out):
    """Shared spmm loop: per slot-window load the expanded message stream,
    build one-hot S per chunk (alternating DVE/Pool), call per_chunk_mm for
    the PE accumulation, then per_window_out."""
    ncw, offs, totc = meta
    sb = [0]

    for win in range(NWIN):
        nw = ncw[win]
        off = int(offs[win])
        G = gpool.tile([P, nw, H], f16, tag="G")
        nc.sync.dma_start(out=G[:], in_=supx[:, off:off + nw, :])
        accs = per_chunk_mm(None, None, -1, nw)    # fresh psum tiles
        for c in range(nw):
            S = spool.tile([P, P], f16, tag="S")
            eng = nc.vector if (sb[0] % 3) < 2 else nc.gpsimd
            sb[0] += 1
            eng.tensor_scalar(
                out=S[:], in0=iota_t[:],
                scalar1=metafull[:, 2 * off + c:2 * off + c + 1],
                scalar2=metafull[:, 2 * off + nw + c:2 * off + nw + c + 1],
                op0=mybir.AluOpType.is_equal, op1=mybir.AluOpType.mult)
            per_chunk_mm(G, S, c, nw, accs)
        per_window_out(win, accs)


def _build_l2(meta):
    """h1T = relu(spmmT(sup1 stream)); s23 = h1 @ W23 (transpose-free)."""
    ncw, offs, totc = meta
    nc = _mk_nc()
    supx = nc.dram_tensor("supx", [P, totc, H1], f16, kind="ExternalInput")
    emeta = nc.dram_tensor("emeta", [P, 2 * totc], f16, kind="ExternalInput")
    W23 = nc.dram_tensor("W23", [H1, H23], f16, kind="ExternalInput")
    iota_h = nc.dram_tensor("iota", [P, P], f16, kind="ExternalInput")
    s23 = nc.dram_tensor("s23", [P, NPAIR, 2, H23], f16, kind="ExternalOutput")

    KC2 = H1 // P                 # 2 feature k-chunks
    with tile.TileContext(nc) as tc:
        with tc.tile_pool(name="const", bufs=1) as cpool, \
             tc.tile_pool(name="spool", bufs=12) as spool, \
             tc.tile_pool(name="gpool", bufs=4) as gpool, \
             tc.tile_pool(name="hpool", bufs=3) as hpool, \
             tc.tile_pool(name="opool", bufs=3) as opool, \
             tc.tile_pool(name="psum", bufs=3, space="PSUM") as psum, \
             tc.tile_pool(name="psum2", bufs=2, space="PSUM") as psum2:
            iota_t = cpool.tile([P, P], f16)
            nc.scalar.dma_start(out=iota_t[:], in_=iota_h[:])
            metah = cpool.tile([P, 2 * totc], f16)
            nc.scalar.dma_start(out=metah[:], in_=emeta[:])
            metafull = cpool.tile([P, 2 * totc], f32)
            nc.vector.tensor_copy(out=metafull[:], in_=metah[:])
            w23c = cpool.tile([P, KC2, H23], f16)
            nc.scalar.dma_start(out=w23c[:],
                                in_=W23[:].rearrange("(k p) n -> p k n", p=P))

            opair_box = [None]

            def per_chunk_mm(G, S, c, nw, accs=None):
                if c == -1:
                    return [psum.tile([P, P], f32, space="PSUM",
                                      name=f"accT{k}", tag=f"accT{k}")
                            for k in range(KC2)]
                for k in range(KC2):
                    nc.tensor.matmul(
                        out=accs[k][:],
                        lhsT=G[:, c, k * P:(k + 1) * P],
                        rhs=S[:],
                        start=(c == 0), stop=(c == nw - 1))

            def per_window_out(win, accs):
                h1T = hpool.tile([P, KC2, P], f16, tag="h1T")
                for k in range(KC2):
                    nc.scalar.activation(out=h1T[:, k, :], in_=accs[k][:],
                                         func=mybir.ActivationFunctionType.Relu)
                ps23 = psum2.tile([P, H23], f32, space="PSUM", tag="ps23")
                for k in range(KC2):
                    nc.tensor.matmul(
                        out=ps23[:],
                        lhsT=h1T[:, k, :],
                        rhs=w23c[:, k, :],
                        start=(k == 0), stop=(k == KC2 - 1))
                if win % 2 == 0:
                    opair_box[0] = opool.tile([P, 2, H23], f16, name="opair", tag="opair")
                opair = opair_box[0]
                nc.scalar.activation(out=opair[:, win % 2, :], in_=ps23[:],
                                     func=mybir.ActivationFunctionType.Copy)
                pb = win // 2
                if win % 2 == 1:
                    nc.scalar.dma_start(out=s23[:, pb, :, :], in_=opair[:])
                elif win == NWIN - 1:
                    nc.scalar.dma_start(out=s23[:, pb, 0, :],
                                        in_=opair[:, 0, :])

            _spmm_windows(nc, spool, psum, gpool, metafull, supx, iota_t,
                          meta, H1, per_chunk_mm, per_window_out)
    nc.compile()
    return nc


def _build_l3(meta):
    """[mu|logvar] = relu(spmm(sup23 stream)); z = eps*exp(logvar)+mu."""
    ncw, offs, totc = meta
    nc = _mk_nc()
    supx = nc.dram_tensor("supx", [P, totc, H23], f16, kind="ExternalInput")
    emeta = nc.dram_tensor("emeta", [P, 2 * totc], f16, kind="ExternalInput")
    iota_h = nc.dram_tensor("iota", [P, P], f16, kind="ExternalInput")
    epss = nc.dram_tensor("epss", [P, NWIN * H2], f16, kind="ExternalInput")
    out3 = nc.dram_tensor("out3", [P, NPAIR, 2, 3 * H2], f16,
                          kind="ExternalOutput")

    with tile.TileContext(nc) as tc:
        with tc.tile_pool(name="const", bufs=1) as cpool, \
             tc.tile_pool(name="spool", bufs=12) as spool, \
             tc.tile_pool(name="gpool", bufs=4) as gpool, \
             tc.tile_pool(name="expool", bufs=3) as expool, \
             tc.tile_pool(name="opool", bufs=3) as opool, \
             tc.tile_pool(name="psum", bufs=4, space="PSUM") as psum:
            iota_t = cpool.tile([P, P], f16)
            nc.scalar.dma_start(out=iota_t[:], in_=iota_h[:])
            metah = cpool.tile([P, 2 * totc], f16)
            nc.scalar.dma_start(out=metah[:], in_=emeta[:])
            metafull = cpool.tile([P, 2 * totc], f32)
            nc.vector.tensor_copy(out=metafull[:], in_=metah[:])
            epsfull = cpool.tile([P, NWIN, H2], f16)
            nc.scalar.dma_start(out=epsfull[:], in_=epss[:])

            opair_box = [None]
            pending = []

            def per_chunk_mm(G, S, c, nw, accs=None):
                if c == -1:
                    return [psum.tile([P, H23], f32, space="PSUM",
                                      name="acc", tag="acc")]
                nc.tensor.matmul(
                    out=accs[0][:],
                    lhsT=S[:],
                    rhs=G[:, c, :],
                    start=(c == 0), stop=(c == nw - 1))

            def tail(win, op, ex_t):
                # deferred one window so the DVE queue never waits on Act
                q = win % 2
                nc.vector.tensor_mul(out=op[:, q, 0:H2], in0=ex_t[:],
                                     in1=epsfull[:, win, :])
                nc.vector.tensor_add(out=op[:, q, 0:H2], in0=op[:, q, 0:H2],
                                     in1=op[:, q, H2:H23])
                pb = win // 2
                if win % 2 == 1:
                    nc.scalar.dma_start(out=out3[:, pb, :, :], in_=op[:])
                elif win == NWIN - 1:
                    nc.scalar.dma_start(out=out3[:, pb, 0, :], in_=op[:, 0, :])

            def per_window_out(win, accs):
                acc = accs[0]
                if win % 2 == 0:
                    opair_box[0] = opool.tile([P, 2, 3 * H2], f16,
                                              name="opair", tag="opair")
                op = opair_box[0]
                q = win % 2
                # o = [z | mu | logvar]
                nc.scalar.activation(out=op[:, q, H2:3 * H2], in_=acc[:],
                                     func=mybir.ActivationFunctionType.Relu)
                ex_t = expool.tile([P, H2], f16, tag="ex")
                nc.scalar.activation(out=ex_t[:], in_=op[:, q, H23:3 * H2],
                                     func=mybir.ActivationFunctionType.Exp)
                pending.append((win, op, ex_t))
                if len(pending) > 1:
                    tail(*pending.pop(0))
                if win == NWIN - 1:
                    while pending:
                        tail(*pending.pop(0))

            _spmm_windows(nc, spool, psum, gpool, metafull, supx, iota_t,
                          meta, H23, per_chunk_mm, per_window_out)
    nc.compile()
    return nc


def _get_progs(key, meta):
    if key not in _PROG_CACHE:
        _PROG_CACHE[key] = (_build_l1(), _build_l2(meta), _build_l3(meta))
    return _PROG_CACHE[key]


# ------------------------------------------------------------------- kernel
def _run_spmd(nc, in_maps, tries=4):
    """run_bass_kernel_spmd with retries: the shared device pool occasionally
    needs a few minutes to recover a wedged worker."""
    import time
    for attempt in range(tries):
        try:
            return run_bass_kernel_spmd(nc, in_maps, core_ids=list(range(M)))
        except Exception:
            if attempt == tries - 1:
                raise
            time.sleep(90)


def kernel(x, W1, W2, W3, edge_weight, eps, edge_src, edge_dst):
    x = np.asarray(x, np.float32)
    W1 = np.asarray(W1, np.float32)
    W23 = np.concatenate([np.asarray(W2, np.float32),
                          np.asarray(W3, np.float32)], axis=1)
    eps = np.asarray(eps, np.float32)

    key, meta, eshards = _prep_edges(edge_src, edge_dst, edge_weight)
    ncw, offs, totc = meta
    nc1, nc2, nc3 = _get_progs(key, meta)

    iota = np.broadcast_to(
        np.arange(P, dtype=np.float16)[None, :], (P, P))

    # ---- L1: support1 shards
    NPAD = NWIN * P
    in1 = []
    for m in range(M):
        xs = np.zeros((NPAD, F_IN), np.float16)
        xs[:NSH] = x[m * NSH:(m + 1) * NSH].astype(np.float16)
        xLm = np.ascontiguousarray(
            xs.reshape(NPAD, KCH, P).transpose(2, 0, 1))   # [128, NPAD, KCH]
        in1.append({"xL": xLm, "W1": W1.astype(np.float16)})
    r1 = _run_spmd(nc1, in1)
    sup1 = np.concatenate(
        [r1.results[m]["s1"].transpose(1, 0, 2).reshape(NPAD, H1)[:NSH]
         for m in range(M)], axis=0)                       # [N, 256] f16

    def unslot(block, m, H):
        """[NWIN*P, H] slot-blocked -> [NSH, H] node-ordered for core m."""
        perm = eshards[m][2]
        out = np.empty((NSH, H), block.dtype)
        for j in range(NWIN):
            wj = int(perm[j])
            r = min(P, NSH - wj * P)
            out[wj * P:wj * P + r] = block[j * P:j * P + r]
        return out

    def toslot(arr, m):
        """[NSH, H] node-ordered -> [NWIN*P, H] slot-blocked for core m."""
        perm = eshards[m][2]
        out = np.zeros((NWIN * P, arr.shape[1]), arr.dtype)
        for j in range(NWIN):
            wj = int(perm[j])
            r = min(P, NSH - wj * P)
            out[j * P:j * P + r] = arr[wj * P:wj * P + r]
        return out

    # ---- L2: h1 + support23 shards
    in2 = [{"supx": _expand(sup1, eshards[m][0], H1),
            "emeta": eshards[m][1],
            "W23": W23.astype(np.float16), "iota": np.asarray(iota)}
           for m in range(M)]
    r2 = _run_spmd(nc2, in2)
    sup23_parts = []
    for m in range(M):
        pr = r2.results[m]["s23"]                  # [128, NPAIR, 2, 128]
        sl = pr.transpose(1, 2, 0, 3).reshape(NPAIR * 2 * P, H23)[:NWIN * P]
        sup23_parts.append(unslot(sl, m, H23))
    sup23 = np.concatenate(sup23_parts, axis=0)    # [N, 128] f16

    # ---- L3: mu, logvar, z shards
    in3 = [{"supx": _expand(sup23, eshards[m][0], H23),
            "emeta": eshards[m][1], "iota": np.asarray(iota),
            "epss": np.ascontiguousarray(
                toslot(eps[m * NSH:(m + 1) * NSH].astype(np.float16), m)
                .reshape(NWIN, P, H2).transpose(1, 0, 2).reshape(P, NWIN * H2))}
           for m in range(M)]
    r3 = _run_spmd(nc3, in3)
    outs = []
    for m in range(M):
        pr = r3.results[m]["out3"]                 # [128, NPAIR, 2, 192]
        sl = pr.transpose(1, 2, 0, 3).reshape(NPAIR * 2 * P, 3 * H2)[:NWIN * P]
        outs.append(unslot(sl, m, 3 * H2))
    full = np.concatenate(outs, axis=0).astype(np.float32)
    z, mu, logvar = full[:, 0:H2], full[:, H2:H23], full[:, H23:3 * H2]
    return (np.ascontiguousarray(z), np.ascontiguousarray(mu),
            np.ascontiguousarray(logvar))


# revision 4
# speedup vs baseline: 1.0145x; 1.0056x over previous
"""GCN-VAE encoder on 8 Trainium2 NeuronCores — v2.

Design (vs v1 baseline): the sparse aggregation's per-edge dma_gather is
replaced by host-side per-edge halo exchange: between launches the host lays
out, per core, the message table sup[src_e] in (slot, chunk, partition) order
so each device reads it as a single sequential stream at full DMA bus rate
(v1's 256-byte gather rows paid a 2x sub-512B descriptor penalty and per-row
descriptor costs; v1 Pool desc-gen is also gone). Segment-sum stays a PE
matmul acc += S^T @ G with one-hot S built from EMETA; S-builds alternate
DVE/Pool to keep both under the DMA roofline. The L2 dense tail computes
h1^T directly (accT = G^T-slices @ S) so h1 @ [W2|W3] needs no transposes.

Launches:
  L1: support1_shard = x_shard @ W1               (node-window pipelined)
  L2: h1T = relu(spmmT(SUPX2)); s23 = h1 @ W23    (per dst-window)
  L3: [mu|logvar] = relu(spmm(SUPX3)); z = eps*exp(logvar)+mu
"""

import sys

for _p in ("/opt/trn_rl_repo", "/root/.axon_site/_ro/trn_rl_repo"):
    if _p not in sys.path:
        sys.path.append(_p)

import numpy as np

import concourse.mybir as mybir
import concourse.tile as tile
from concourse import bacc
from concourse.bass_utils import run_bass_kernel_spmd

# ---- problem constants (hardcoded per harness contract) ----
N, E, F_IN, H1, H2 = 50000, 1600000, 512, 256, 64
H23 = 2 * H2                      # concat(mu, logvar) feature width
M = 8                             # cores
NSH = N // M                      # nodes per core
P = 128                           # partitions / window size / edge chunk
NWIN = (NSH + P - 1) // P         # dst windows per core (49)
KCH = F_IN // P                   # k-chunks for layer-1 matmul (4)
NPAIR = (NWIN + 1) // 2

f32 = mybir.dt.float32
f16 = mybir.dt.float16

_PROG_CACHE: dict = {}


# ---------------------------------------------------------------- host prep
def _prep_edges(edge_src, edge_dst, edge_weight):
    """Partition edges by dst shard, window-sort, slot-balance; emit per-core
    (SRC slot-ordered edge sources, EMETA one-hot build scalars, perm)."""
    edge_src = np.asarray(edge_src).astype(np.int64)
    edge_dst = np.asarray(edge_dst).astype(np.int64)
    edge_weight = np.asarray(edge_weight).astype(np.float32)

    percore = []
    cnts = np.zeros((M, NWIN), np.int64)
    perms = []
    for m in range(M):
        sel = (edge_dst >= m * NSH) & (edge_dst < (m + 1) * NSH)
        d = edge_dst[sel] - m * NSH
        s = edge_src[sel]
        w = edge_weight[sel]
        win0 = d >> 7
        # slot-balance: rank windows by edge count so slot j holds each
        # core's j-th-largest window -> cross-core max padding shrinks
        wcnt = np.bincount(win0, minlength=NWIN)
        perm = np.argsort(-wcnt, kind="stable")           # slot -> window
        inv = np.empty(NWIN, np.int64)
        inv[perm] = np.arange(NWIN)                       # window -> slot
        perms.append(perm)
        win = inv[win0]                                   # slot index per edge
        order = np.argsort(win, kind="stable")
        d, s, w, win = d[order], s[order], w[order], win[order]
        dloc = d - (perm[win] << 7)                       # dst_local in window
        cnts[m] = np.bincount(win, minlength=NWIN)
        percore.append((dloc, s, w, win))

    ncw = np.maximum(1, -(-cnts.max(axis=0) // P))        # chunks per slot
    offs = np.concatenate([[0], np.cumsum(ncw)])
    totc = int(offs[-1])

    shards = []
    for m in range(M):
        dloc, s, w, win = percore[m]
        DSTL = np.full((P, totc), -1.0, np.float32)
        WGT = np.zeros((P, totc), np.float32)
        SRC = np.full(totc * P, -1, np.int64)
        starts = np.concatenate([[0], np.cumsum(cnts[m])])[:-1]
        j = np.arange(len(dloc)) - starts[win]            # rank within slot
        rows = j % P
        cols = offs[win] + j // P
        DSTL[rows, cols] = dloc
        WGT[rows, cols] = w
        SRC[cols * P + rows] = s
        EMETA = np.empty((P, 2 * totc), np.float16)
        for t in range(NWIN):
            o, nw = int(offs[t]), int(ncw[t])
            EMETA[:, 2 * o:2 * o + nw] = DSTL[:, o:o + nw]
            EMETA[:, 2 * o + nw:2 * (o + nw)] = WGT[:, o:o + nw]
        shards.append((SRC, EMETA, perms[m]))

    key = tuple(int(v) for v in ncw)
    meta = (tuple(int(v) for v in ncw), offs, totc)
    return key, meta, shards


def _expand(sup_full, SRC, H):
    """[N, H] table -> [128, totc, H] slot-ordered per-edge message stream."""
    out = np.zeros((SRC.shape[0], H), sup_full.dtype)
    valid = SRC >= 0
    out[valid] = sup_full[SRC[valid]]
    return np.ascontiguousarray(
        out.reshape(-1, P, H).transpose(1, 0, 2))


# ------------------------------------------------------------- bass builders
def _mk_nc():
    return bacc.Bacc("TRN2", target_bir_lowering=False, debug=False)


def _build_l1():
    """s1[128, NWIN, 256] = x_shard @ W1, span-pipelined."""
    nc = _mk_nc()
    xL = nc.dram_tensor("xL", [P, NWIN * P, KCH], f16, kind="ExternalInput")
    W1 = nc.dram_tensor("W1", [F_IN, H1], f16, kind="ExternalInput")
    s1 = nc.dram_tensor("s1", [P, NWIN, H1], f16, kind="ExternalOutput")

    SPANS = [1, 2, 4] + [7] * 6   # ramped: first matmuls start early
    assert sum(SPANS) == NWIN
    with tile.TileContext(nc) as tc:
        with tc.tile_pool(name="const", bufs=1) as cpool, \
             tc.tile_pool(name="xin", bufs=4) as xpool, \
             tc.tile_pool(name="out", bufs=3) as opool, \
             tc.tile_pool(name="psum", bufs=4, space="PSUM") as psum:
            w1c = cpool.tile([P, KCH, H1], f16)
            nc.sync.dma_start(out=w1c[:],
                              in_=W1[:].rearrange("(k p) n -> p k n", p=P))
            a0 = 0
            for sp, nsw in enumerate(SPANS):
                a = a0 * P
                xsp = xpool.tile([P, nsw * P, KCH], f16, name="xsp", tag="xsp")
                nc.sync.dma_start(out=xsp[:],
                                  in_=xL[:, a:a + nsw * P, :])
                osp = opool.tile([P, nsw, H1], f16, name="osp", tag="osp")
                for t in range(nsw):
                    acc = psum.tile([P, H1], f32, space="PSUM",
                                    name="acc", tag="acc")
                    for k in range(KCH):
                        nc.tensor.matmul(
                            out=acc[:],
                            lhsT=xsp[:, t * P:(t + 1) * P, k],
                            rhs=w1c[:, k, :],
                            start=(k == 0), stop=(k == KCH - 1))
                    nc.vector.tensor_copy(out=osp[:, t, :], in_=acc[:])
                nc.scalar.dma_start(out=s1[:, a0:a0 + nsw, :], in_=osp[:])
                a0 += nsw
    nc.compile()
    return nc


def _spmm_windows(nc, spool, psum, gpool, metafull, supx, iota_t, meta,
                  H, per_chunk_mm, per_window_out):
    """Shared spmm loop: per slot-window load the expanded message stream,
    build one-hot S per chunk (alternating DVE/Pool), call per_chunk_mm for
    the PE accumulation, then per_window_out."""
    ncw, offs, totc = meta
    sb = [0]

    for win in range(NWIN):
        nw = ncw[win]
        off = int(offs[win])
        G = gpool.tile([P, nw, H], f16, name="G", tag="G")
        nc.sync.dma_start(out=G[:], in_=supx[:, off:off + nw, :])

        def getg(c, _g=G):
            return (_g, c)
        accs = per_chunk_mm(None, None, -1, nw)    # fresh psum tiles
        for c in range(nw):
            S = spool.tile([P, P], f16, name="S", tag="S")
            eng = nc.vector if (sb[0] % 3) < 2 else nc.gpsimd
            sb[0] += 1
            eng.tensor_scalar(
                out=S[:], in0=iota_t[:],
                scalar1=metafull[:, 2 * off + c:2 * off + c + 1],
                scalar2=metafull[:, 2 * off + nw + c:2 * off + nw + c + 1],
                op0=mybir.AluOpType.is_equal, op1=mybir.AluOpType.mult)
            per_chunk_mm(getg, S, c, nw, accs)
        per_window_out(win, accs)


def _build_l2(meta):
    """h1T = relu(spmmT(sup1 stream)); s23 = h1 @ W23 (transpose-free)."""
    ncw, offs, totc = meta
    nc = _mk_nc()
    supx = nc.dram_tensor("supx", [P, totc, H1], f16, kind="ExternalInput")
    emeta = nc.dram_tensor("emeta", [P, 2 * totc], f16, kind="ExternalInput")
    W23 = nc.dram_tensor("W23", [H1, H23], f16, kind="ExternalInput")
    iota_h = nc.dram_tensor("iota", [P, P], f16, kind="ExternalInput")
    s23 = nc.dram_tensor("s23", [P, NPAIR, 2, H23], f16, kind="ExternalOutput")

    KC2 = H1 // P                 # 2 feature k-chunks
    with tile.TileContext(nc) as tc:
        with tc.tile_pool(name="const", bufs=1) as cpool, \
             tc.tile_pool(name="spool", bufs=12) as spool, \
             tc.tile_pool(name="gpool", bufs=4) as gpool, \
             tc.tile_pool(name="hpool", bufs=3) as hpool, \
             tc.tile_pool(name="opool", bufs=3) as opool, \
             tc.tile_pool(name="psum", bufs=3, space="PSUM") as psum, \
             tc.tile_pool(name="psum2", bufs=2, space="PSUM") as psum2:
            iota_t = cpool.tile([P, P], f16)
            nc.scalar.dma_start(out=iota_t[:], in_=iota_h[:])
            metah = cpool.tile([P, 2 * totc], f16)
            nc.scalar.dma_start(out=metah[:], in_=emeta[:])
            metafull = cpool.tile([P, 2 * totc], f32)
            nc.vector.tensor_copy(out=metafull[:], in_=metah[:])
            w23c = cpool.tile([P, KC2, H23], f16)
            nc.scalar.dma_start(out=w23c[:],
                                in_=W23[:].rearrange("(k p) n -> p k n", p=P))

            opair_box = [None]

            def per_chunk_mm(getg, S, c, nw, accs=None):
                if c == -1:
                    return [psum.tile([P, P], f32, space="PSUM",
                                      name=f"accT{k}", tag=f"accT{k}")
                            for k in range(KC2)]
                G, lc = getg(c)
                for k in range(KC2):
                    nc.tensor.matmul(
                        out=accs[k][:],
                        lhsT=G[:, lc, k * P:(k + 1) * P],
                        rhs=S[:],
                        start=(c == 0), stop=(c == nw - 1))

            def per_window_out(win, accs):
                h1T = hpool.tile([P, KC2, P], f16, tag="h1T")
                for k in range(KC2):
                    nc.scalar.activation(out=h1T[:, k, :], in_=accs[k][:],
                                         func=mybir.ActivationFunctionType.Relu)
                ps23 = psum2.tile([P, H23], f32, space="PSUM", tag="ps23")
                for k in range(KC2):
                    nc.tensor.matmul(
                        out=ps23[:],
                        lhsT=h1T[:, k, :],
                        rhs=w23c[:, k, :],
                        start=(k == 0), stop=(k == KC2 - 1))
                if win % 2 == 0:
                    opair_box[0] = opool.tile([P, 2, H23], f16, name="opair", tag="opair")
                opair = opair_box[0]
                nc.scalar.activation(out=opair[:, win % 2, :], in_=ps23[:],
                                     func=mybir.ActivationFunctionType.Copy)
                pb = win // 2
                if win % 2 == 1:
                    nc.scalar.dma_start(out=s23[:, pb, :, :], in_=opair[:])
                elif win == NWIN - 1:
                    nc.scalar.dma_start(out=s23[:, pb, 0, :],
                                        in_=opair[:, 0, :])

            _spmm_windows(nc, spool, psum, gpool, metafull, supx, iota_t,
                          meta, H1, per_chunk_mm, per_window_out)
    nc.compile()
    return nc


def _build_l3(meta):
    """[mu|logvar] = relu(spmm(sup23 stream)); z = eps*exp(logvar)+mu."""
    ncw, offs, totc = meta
    nc = _mk_nc()
    supx = nc.dram_tensor("supx", [P, totc, H23], f16, kind="ExternalInput")
    emeta = nc.dram_tensor("emeta", [P, 2 * totc], f16, kind="ExternalInput")
    iota_h = nc.dram_tensor("iota", [P, P], f16, kind="ExternalInput")
    epss = nc.dram_tensor("epss", [P, NWIN * H2], f16, kind="ExternalInput")
    out3 = nc.dram_tensor("out3", [P, NPAIR, 2, 3 * H2], f16,
                          kind="ExternalOutput")

    with tile.TileContext(nc) as tc:
        with tc.tile_pool(name="const", bufs=1) as cpool, \
             tc.tile_pool(name="spool", bufs=12) as spool, \
             tc.tile_pool(name="gpool", bufs=4) as gpool, \
             tc.tile_pool(name="expool", bufs=3) as expool, \
             tc.tile_pool(name="opool", bufs=3) as opool, \
             tc.tile_pool(name="psum", bufs=6, space="PSUM") as psum:
            iota_t = cpool.tile([P, P], f16)
            nc.scalar.dma_start(out=iota_t[:], in_=iota_h[:])
            metah = cpool.tile([P, 2 * totc], f16)
            nc.scalar.dma_start(out=metah[:], in_=emeta[:])
            metafull = cpool.tile([P, 2 * totc], f32)
            nc.vector.tensor_copy(out=metafull[:], in_=metah[:])
            epsfull = cpool.tile([P, NWIN, H2], f16)
            nc.scalar.dma_start(out=epsfull[:], in_=epss[:])

            opair_box = [None]
            pending = []

            def per_chunk_mm(getg, S, c, nw, accs=None):
                if c == -1:
                    return [psum.tile([P, H23], f32, space="PSUM",
                                      name="acc", tag="acc")]
                G, lc = getg(c)
                nc.tensor.matmul(
                    out=accs[0][:],
                    lhsT=S[:],
                    rhs=G[:, lc, :],
                    start=(c == 0), stop=(c == nw - 1))

            def tail(win, op, ex_t):
                # deferred one window so the DVE queue never waits on Act
                q = win % 2
                nc.vector.tensor_mul(out=op[:, q, 0:H2], in0=ex_t[:],
                                     in1=epsfull[:, win, :])
                nc.vector.tensor_add(out=op[:, q, 0:H2], in0=op[:, q, 0:H2],
                                     in1=op[:, q, H2:H23])
                pb = win // 2
                if win % 2 == 1:
                    nc.scalar.dma_start(out=out3[:, pb, :, :], in_=op[:])
                elif win == NWIN - 1:
                    nc.scalar.dma_start(out=out3[:, pb, 0, :], in_=op[:, 0, :])

            def per_window_out(win, accs):
                acc = accs[0]
                if win % 2 == 0:
                    opair_box[0] = opool.tile([P, 2, 3 * H2], f16,
                                              name="opair", tag="opair")
                op = opair_box[0]
                q = win % 2
                # o = [z | mu | logvar]
                nc.scalar.activation(out=op[:, q, H2:3 * H2], in_=acc[:],
                                     func=mybir.ActivationFunctionType.Relu)
                ex_t = expool.tile([P, H2], f16, tag="ex")
                nc.scalar.activation(out=ex_t[:], in_=op[:, q, H23:3 * H2],
                                     func=mybir.ActivationFunctionType.Exp)
                pending.append((win, op, ex_t))
                if len(pending) > 1:
                    tail(*pending.pop(0))
                if win == NWIN - 1:
                    while pending:
                        tail(*pending.pop(0))

            _spmm_windows(nc, spool, psum, gpool, metafull, supx, iota_t,
                          meta, H23, per_chunk_mm, per_window_out)
    nc.compile()
    return nc


def _get_progs(key, meta):
    if key not in _PROG_CACHE:
        _PROG_CACHE[key] = (_build_l1(), _build_l2(meta), _build_l3(meta))
    return _PROG_CACHE[key]


# ------------------------------------------------------------------- kernel
def _run_spmd(nc, in_maps, tries=4):
    """run_bass_kernel_spmd with retries: the shared device pool occasionally
    needs a few minutes to recover a wedged worker."""
    import time
    for attempt in range(tries):
        try:
            return run_bass_kernel_spmd(nc, in_maps, core_ids=list(range(M)))
        except Exception:
            if attempt == tries - 1:
                raise
            time.sleep(90)


def kernel(x, W1, W2, W3, edge_weight, eps, edge_src, edge_dst):
    x = np.asarray(x, np.float32)
    W1 = np.asarray(W1, np.float32)
    W23 = np.concatenate([np.asarray(W2, np.float32),
                          np.asarray(W3, np.float32)], axis=1)
    eps = np.asarray(eps, np.float32)

    key, meta, eshards = _prep_edges(edge_src, edge_dst, edge_weight)
    ncw, offs, totc = meta
    nc1, nc2, nc3 = _get_progs(key, meta)

    iota = np.broadcast_to(
        np.arange(P, dtype=np.float16)[None, :], (P, P))

    # ---- L1: support1 shards
    NPAD = NWIN * P
    in1 = []
    for m in range(M):
        xs = np.zeros((NPAD, F_IN), np.float16)
        xs[:NSH] = x[m * NSH:(m + 1) * NSH].astype(np.float16)
        xLm = np.ascontiguousarray(
            xs.reshape(NPAD, KCH, P).transpose(2, 0, 1))   # [128, NPAD, KCH]
        in1.append({"xL": xLm, "W1": W1.astype(np.float16)})
    r1 = _run_spmd(nc1, in1)
    sup1 = np.concatenate(
        [r1.results[m]["s1"].transpose(1, 0, 2).reshape(NPAD, H1)[:NSH]
         for m in range(M)], axis=0)                       # [N, 256] f16

    def unslot(block, m, H):
        """[NWIN*P, H] slot-blocked -> [NSH, H] node-ordered for core m."""
        perm = eshards[m][2]
        out = np.empty((NSH, H), block.dtype)
        for j in range(NWIN):
            wj = int(perm[j])
            r = min(P, NSH - wj * P)
            out[wj * P:wj * P + r] = block[j * P:j * P + r]
        return out

    def toslot(arr, m):
        """[NSH, H] node-ordered -> [NWIN*P, H] slot-blocked for core m."""
        perm = eshards[m][2]
        out = np.zeros((NWIN * P, arr.shape[1]), arr.dtype)
        for j in range(NWIN):
            wj = int(perm[j])
            r = min(P, NSH - wj * P)
            out[j * P:j * P + r] = arr[wj * P:wj * P + r]
        return out

    # ---- L2: h1 + support23 shards
    in2 = [{"supx": _expand(sup1, eshards[m][0], H1),
            "emeta": eshards[m][1],
            "W23": W23.astype(np.float16), "iota": np.asarray(iota)}
           for m in range(M)]
    r2 = _run_spmd(nc2, in2)
    sup23_parts = []
    for m in range(M):
        pr = r2.results[m]["s23"]                  # [128, NPAIR, 2, 128]
        sl = pr.transpose(1, 2, 0, 3).reshape(NPAIR * 2 * P, H23)[:NWIN * P]
        sup23_parts.append(unslot(sl, m, H23))
    sup23 = np.concatenate(sup23_parts, axis=0)    # [N, 128] f16

    # ---- L3: mu, logvar, z shards
    in3 = [{"supx": _expand(sup23, eshards[m][0], H23),
            "emeta": eshards[m][1], "iota": np.asarray(iota),
            "epss": np.ascontiguousarray(
                toslot(eps[m * NSH:(m + 1) * NSH].astype(np.float16), m)
                .reshape(NWIN, P, H2).transpose(1, 0, 2).reshape(P, NWIN * H2))}
           for m in range(M)]
    r3 = _run_spmd(nc3, in3)
    outs = []
    for m in range(M):
        pr = r3.results[m]["out3"]                 # [128, NPAIR, 2, 192]
        sl = pr.transpose(1, 2, 0, 3).reshape(NPAIR * 2 * P, 3 * H2)[:NWIN * P]
        outs.append(unslot(sl, m, 3 * H2))
    full = np.concatenate(outs, axis=0).astype(np.float32)
    z, mu, logvar = full[:, 0:H2], full[:, H2:H23], full[:, H23:3 * H2]
    return (np.ascontiguousarray(z), np.ascontiguousarray(mu),
            np.ascontiguousarray(logvar))
